# revision 11
# baseline (speedup 1.0000x reference)
"""Trainium2 Bass kernel for nn_BCE_for_non_zero.

Reference computation (B=2e6 rows, C=14 labels, 4 label-groups):
    bce  = max(x,0) - x*t + log1p(exp(-|x|))          # = softplus(x) - x*t
    s_t  = per-row sums of t within each label group
    mask = 1 for group-0 labels, else (s_t[group] > 0)
    out  = mean(bce * mask)

Rewriting the masked grand total per row b:
    total[b] = sum_c bce[b,c] - sum_{g!=0, s_t_g==0} sum_{c in g} bce[b,c]
and when s_t_g == 0 every t in the group is 0, so bce there is just
softplus(x).  Hence per row:
    total = [sum_c softplus(x)] - [sum_c x*t] - sum_{g!=0} drop_g * s_sp_g
with drop_g = (s_t_g == 0), s_sp_g = group sums of softplus(x).

Per-core mapping (pure data parallel over rows, 8 cores):
  - tile rows as [128 partitions, K rows/partition, 14], per-partition
    contiguous DMA
  - ACT: softplus full pass (in-place over x) with fused free-dim row-sum
    (accum_out)
  - DVE: fused multiply-reduce for -sum(x*t); strided adds for per-group
    softplus sums; fused multiply-reduce for the dropped-group correction
  - GPSIMD: per-group target sums (keeps DVE under the DMA roofline)
  - ACT: drop mask via relu(1 - s_t)
Partial sums leave the chip as one [128, n_tiles] f32 tensor per core;
the host reduces them in f64.
"""

import numpy as np

C = 14
P = 128
NUM_GROUPS = 4
N_CORES = 8
MAX_K = 651  # rows per partition per tile; 3 tiles cover 1953 blocks/core

_prog_cache = {}


def _plan_tiles(rows, max_k=MAX_K):
    """Split `rows` into (row0, p, k) tiles: full [128, k] tiles + a tail."""
    nb, tail = divmod(rows, P)
    tiles = []
    row0 = 0
    if nb > 0:
        n_full = -(-nb // max_k)
        base, rem = divmod(nb, n_full)
        for i in range(n_full):
            k = base + (1 if i < rem else 0)
            tiles.append((row0, P, k))
            row0 += P * k
    if tail:
        tiles.append((row0, tail, 1))
    return tiles


def _group_cols(groups):
    """Columns of each non-empty group with id != 0."""
    return [
        cols
        for g in range(1, NUM_GROUPS)
        if (cols := [c for c in range(C) if groups[c] == g])
    ]


def build_program(rows, groups, st_engine="gpsimd"):
    import concourse.bacc as bacc
    import concourse.mybir as mybir
    from concourse.tile import TileContext

    f32 = mybir.dt.float32
    mult = mybir.AluOpType.mult
    add = mybir.AluOpType.add

    gcols = _group_cols(groups)
    G = len(gcols)
    tiles = _plan_tiles(rows)
    n_tiles = len(tiles)

    nc = bacc.Bacc("TRN2", target_bir_lowering=False, debug=False)
    x_d = nc.dram_tensor("x", [rows, C], f32, kind="ExternalInput")
    t_d = nc.dram_tensor("t", [rows, C], f32, kind="ExternalInput")
    out_d = nc.dram_tensor("out", [P, n_tiles], f32, kind="ExternalOutput")

    with TileContext(nc) as tc:
        with (
            tc.tile_pool(name="big", bufs=2) as big,
            tc.tile_pool(name="small", bufs=2) as small,
            tc.tile_pool(name="accp", bufs=1) as accp,
        ):
            acc = accp.tile([P, n_tiles], f32, tag="acc")
            nc.vector.memset(acc[:, :], 0.0)

            for j, (row0, p, k) in enumerate(tiles):
                kc = k * C
                xt = big.tile([P, kc], f32, tag="x")
                tt = big.tile([P, kc], f32, tag="t")
                xv = x_d.ap()[row0 : row0 + p * k, :].rearrange(
                    "(p k) c -> p (k c)", p=p
                )
                tv = t_d.ap()[row0 : row0 + p * k, :].rearrange(
                    "(p k) c -> p (k c)", p=p
                )
                nc.sync.dma_start(out=xt[:p, :], in_=xv)
                nc.sync.dma_start(out=tt[:p, :], in_=tv)

                x3 = xt[:p, :].rearrange("p (k c) -> p k c", c=C)
                t3 = tt[:p, :].rearrange("p (k c) -> p k c", c=C)

                sigA = small.tile([P, 1], f32, tag="sigA")
                sigB = small.tile([P, 1], f32, tag="sigB")

                if G:
                    st = small.tile([P, G * k], f32, tag="st")
                    ssp = small.tile([P, G * k], f32, tag="ssp")
                    st3 = st[:p, :].rearrange("p (g k) -> p g k", g=G)
                    ssp3 = ssp[:p, :].rearrange("p (g k) -> p g k", g=G)

                    # (a) per-group target sums (t still pristine)
                    eng = getattr(nc, st_engine)
                    for gi, cols in enumerate(gcols):
                        dst = st3[:, gi, :]
                        if len(cols) == 1:
                            eng.tensor_copy(dst, t3[:, :, cols[0]])
                        else:
                            eng.tensor_add(
                                out=dst, in0=t3[:, :, cols[0]], in1=t3[:, :, cols[1]]
                            )
                            for cx in cols[2:]:
                                eng.tensor_add(out=dst, in0=dst, in1=t3[:, :, cx])

                # (b) tt <- (x * -1) * t, sigA = row sums of that.
                # (tensor_tensor_reduce is unsupported by the runtime here;
                # scalar_tensor_tensor has the same fused sum output.)
                nc.vector.scalar_tensor_tensor(
                    out=tt[:p, :],
                    in0=xt[:p, :],
                    scalar=-1.0,
                    in1=tt[:p, :],
                    op0=mult,
                    op1=mult,
                    accum_out=sigA[:p, :],
                )

                # (c) xt <- softplus(xt) = Ln(Exp(x) + 1), sigB = row sums.
                # No softplus act table exists; exp+ln share one table set
                # and the "+1" is the Ln op's free input bias.
                nc.scalar.activation(
                    out=xt[:p, :],
                    in_=xt[:p, :],
                    func=mybir.ActivationFunctionType.Exp,
                )
                nc.scalar.activation(
                    out=xt[:p, :],
                    in_=xt[:p, :],
                    func=mybir.ActivationFunctionType.Ln,
                    bias=1.0,
                    accum_out=sigB[:p, :],
                )

                if G:
                    # (d) per-group softplus sums
                    for gi, cols in enumerate(gcols):
                        dst = ssp3[:, gi, :]
                        if len(cols) == 1:
                            nc.vector.tensor_copy(dst, x3[:, :, cols[0]])
                        else:
                            nc.vector.tensor_add(
                                out=dst, in0=x3[:, :, cols[0]], in1=x3[:, :, cols[1]]
                            )
                            for cx in cols[2:]:
                                nc.vector.tensor_add(out=dst, in0=dst, in1=x3[:, :, cx])

                    # (e) st <- relu(1 - st)  == (s_t == 0) since s_t in {0,1,...}
                    nc.scalar.activation(
                        out=st[:p, :],
                        in_=st[:p, :],
                        func=mybir.ActivationFunctionType.Relu,
                        bias=1.0,
                        scale=-1.0,
                    )

                    # (f) ssp <- (drop * -1) * ssp, sigC = row sums of that
                    sigC = small.tile([P, 1], f32, tag="sigC")
                    nc.vector.scalar_tensor_tensor(
                        out=ssp[:p, :],
                        in0=st[:p, :],
                        scalar=-1.0,
                        in1=ssp[:p, :],
                        op0=mult,
                        op1=mult,
                        accum_out=sigC[:p, :],
                    )
                    nc.vector.tensor_add(
                        out=sigA[:p, :], in0=sigA[:p, :], in1=sigC[:p, :]
                    )

                # (g) per-tile partial: acc[:, j] = sigB + sigA (+ sigC)
                nc.vector.tensor_add(
                    out=acc[:p, j : j + 1], in0=sigB[:p, :], in1=sigA[:p, :]
                )

            nc.sync.dma_start(out=out_d.ap(), in_=acc[:, :])

    nc.compile()
    return nc


def run(inputs, targets, groups, trace=False):
    """Returns (loss, exec_time_ns or None)."""
    from concourse import bass_utils

    B = inputs.shape[0]
    assert inputs.shape[1] == C and B % N_CORES == 0
    rows = B // N_CORES

    key = (rows, tuple(int(v) for v in groups))
    if key not in _prog_cache:
        _prog_cache[key] = build_program(rows, key[1])
    nc = _prog_cache[key]

    x = np.ascontiguousarray(inputs, dtype=np.float32)
    t = np.ascontiguousarray(targets, dtype=np.float32)
    in_maps = [
        {
            "x": x[c * rows : (c + 1) * rows],
            "t": t[c * rows : (c + 1) * rows],
        }
        for c in range(N_CORES)
    ]
    res = bass_utils.run_bass_kernel_spmd(
        nc, in_maps, core_ids=list(range(N_CORES)), trace=trace
    )
    total = sum(float(r["out"].astype(np.float64).sum()) for r in res.results)
    return np.float32(total / (B * C)), res.exec_time_ns


def kernel(inputs, targets, groups):
    return run(inputs, targets, groups)[0]


# revision 12
# speedup vs baseline: 1.1852x; 1.1852x over previous
"""Trainium2 Bass kernel for nn_BCE_for_non_zero.

Reference computation (B=2e6 rows, C=14 labels, 4 label-groups):
    bce  = max(x,0) - x*t + log1p(exp(-|x|))          # = softplus(x) - x*t
    s_t  = per-row sums of t within each label group
    mask = 1 for group-0 labels, else (s_t[group] > 0)
    out  = mean(bce * mask)

Rewriting the masked grand total per row b:
    total[b] = sum_c bce[b,c] - sum_{g!=0, s_t_g==0} sum_{c in g} bce[b,c]
and when s_t_g == 0 every t in the group is 0, so bce there is just
softplus(x).  Hence per row:
    total = [sum_c softplus(x)] - [sum_c x*t] - sum_{g!=0} drop_g * s_sp_g
with drop_g = (s_t_g == 0), s_sp_g = group sums of softplus(x).

Per-core mapping (pure data parallel over rows, 8 cores):
  - tile rows as [128 partitions, K rows/partition, 14]; per-partition
    contiguous DMA; SWDGE cast-DMA converts f32 DRAM -> bf16 SBUF tiles
    (halves SBUF footprint so independent buffers fit and engines overlap)
  - ACT: softplus as Ln(Exp(x)+1) (2 passes, one act-table set) with the
    fused free-dim row-sum (accum_out) on the Ln
  - DVE: fused multiply-reduce scalar_tensor_tensor for -sum(x*t); strided
    adds for per-group softplus sums; is_equal for the drop mask; fused
    multiply-reduce for the dropped-group correction
  - GPSIMD: per-group target sums (parallel with DVE/ACT)
Partial sums leave the chip as one [128, n_tiles] f32 tensor per core;
the host reduces them in f64.
"""

import numpy as np

C = 14
P = 128
NUM_GROUPS = 4
N_CORES = 8
MAX_K = 651  # rows per partition per tile; 3 tiles cover 1953 blocks/core

_prog_cache = {}


def _plan_tiles(rows, max_k=MAX_K):
    """Split `rows` into (row0, p, k) tiles: full [128, k] tiles + a tail."""
    nb, tail = divmod(rows, P)
    tiles = []
    row0 = 0
    if nb > 0:
        n_full = -(-nb // max_k)
        base, rem = divmod(nb, n_full)
        for i in range(n_full):
            k = base + (1 if i < rem else 0)
            tiles.append((row0, P, k))
            row0 += P * k
    if tail:
        tiles.append((row0, tail, 1))
    return tiles


def _group_cols(groups):
    """Columns of each non-empty group with id != 0."""
    return [
        cols
        for g in range(1, NUM_GROUPS)
        if (cols := [c for c in range(C) if groups[c] == g])
    ]


def build_program(rows, groups, st_engine="gpsimd", sbuf_dtype="bf16"):
    import concourse.bacc as bacc
    import concourse.mybir as mybir
    from concourse.tile import TileContext

    f32 = mybir.dt.float32
    bdt = mybir.dt.bfloat16 if sbuf_dtype == "bf16" else f32
    mult = mybir.AluOpType.mult
    add = mybir.AluOpType.add
    is_equal = mybir.AluOpType.is_equal

    gcols = _group_cols(groups)
    G = len(gcols)
    tiles = _plan_tiles(rows)
    n_tiles = len(tiles)

    nc = bacc.Bacc("TRN2", target_bir_lowering=False, debug=False)
    x_d = nc.dram_tensor("x", [rows, C], f32, kind="ExternalInput")
    t_d = nc.dram_tensor("t", [rows, C], f32, kind="ExternalInput")
    out_d = nc.dram_tensor("out", [P, n_tiles], f32, kind="ExternalOutput")

    cast = bdt != f32
    dma_in = nc.gpsimd if cast else nc.sync

    with TileContext(nc) as tc:
        with (
            tc.tile_pool(name="big", bufs=2) as big,
            tc.tile_pool(name="small", bufs=2) as small,
            tc.tile_pool(name="accp", bufs=1) as accp,
        ):
            acc = accp.tile([P, n_tiles], f32, tag="acc")
            nc.vector.memset(acc[:, :], 0.0)

            for j, (row0, p, k) in enumerate(tiles):
                kc = k * C
                xt = big.tile([P, kc], bdt, tag="x")
                tt = big.tile([P, kc], bdt, tag="t")
                yt = big.tile([P, kc], bdt, tag="y")
                jk = big.tile([P, kc], bdt, tag="junk")
                xv = x_d.ap()[row0 : row0 + p * k, :].rearrange(
                    "(p k) c -> p (k c)", p=p
                )
                tv = t_d.ap()[row0 : row0 + p * k, :].rearrange(
                    "(p k) c -> p (k c)", p=p
                )
                dma_in.dma_start(out=xt[:p, :], in_=xv)
                dma_in.dma_start(out=tt[:p, :], in_=tv)

                y3 = yt[:p, :].rearrange("p (k c) -> p k c", c=C)
                t3 = tt[:p, :].rearrange("p (k c) -> p k c", c=C)

                sigA = small.tile([P, 1], f32, tag="sigA")
                sigB = small.tile([P, 1], f32, tag="sigB")

                if G:
                    st = small.tile([P, G * k], bdt, tag="st")
                    ssp = small.tile([P, G * k], bdt, tag="ssp")
                    st3 = st[:p, :].rearrange("p (g k) -> p g k", g=G)
                    ssp3 = ssp[:p, :].rearrange("p (g k) -> p g k", g=G)

                    # (a) per-group target sums (gpsimd; parallel with b/c)
                    eng = getattr(nc, st_engine)
                    for gi, cols in enumerate(gcols):
                        dst = st3[:, gi, :]
                        if len(cols) == 1:
                            eng.tensor_copy(dst, t3[:, :, cols[0]])
                        else:
                            eng.tensor_add(
                                out=dst, in0=t3[:, :, cols[0]], in1=t3[:, :, cols[1]]
                            )
                            for cx in cols[2:]:
                                eng.tensor_add(out=dst, in0=dst, in1=t3[:, :, cx])

                # (b) jk <- (x * -1) * t, sigA = row sums of that
                nc.vector.scalar_tensor_tensor(
                    out=jk[:p, :],
                    in0=xt[:p, :],
                    scalar=-1.0,
                    in1=tt[:p, :],
                    op0=mult,
                    op1=mult,
                    accum_out=sigA[:p, :],
                )

                # (c) y <- softplus(x) = Ln(Exp(x) + 1), sigB = row sums.
                # exp+ln share one act-table set; "+1" is Ln's input bias.
                nc.scalar.activation(
                    out=yt[:p, :],
                    in_=xt[:p, :],
                    func=mybir.ActivationFunctionType.Exp,
                )
                nc.scalar.activation(
                    out=yt[:p, :],
                    in_=yt[:p, :],
                    func=mybir.ActivationFunctionType.Ln,
                    bias=1.0,
                    accum_out=sigB[:p, :],
                )

                if G:
                    # (d) per-group softplus sums
                    for gi, cols in enumerate(gcols):
                        dst = ssp3[:, gi, :]
                        if len(cols) == 1:
                            nc.vector.tensor_copy(dst, y3[:, :, cols[0]])
                        else:
                            nc.vector.tensor_add(
                                out=dst, in0=y3[:, :, cols[0]], in1=y3[:, :, cols[1]]
                            )
                            for cx in cols[2:]:
                                nc.vector.tensor_add(out=dst, in0=dst, in1=y3[:, :, cx])

                    # (e) st <- (st == 0)  (drop mask; contiguous, cheap)
                    nc.vector.tensor_scalar(
                        out=st[:p, :],
                        in0=st[:p, :],
                        scalar1=0.0,
                        scalar2=None,
                        op0=is_equal,
                    )

                    # (f) ssp <- (drop * -1) * ssp, sigC = row sums of that
                    sigC = small.tile([P, 1], f32, tag="sigC")
                    nc.vector.scalar_tensor_tensor(
                        out=ssp[:p, :],
                        in0=st[:p, :],
                        scalar=-1.0,
                        in1=ssp[:p, :],
                        op0=mult,
                        op1=mult,
                        accum_out=sigC[:p, :],
                    )
                    nc.vector.tensor_add(
                        out=sigA[:p, :], in0=sigA[:p, :], in1=sigC[:p, :]
                    )

                # (g) per-tile partial: acc[:, j] = sigB + sigA (+ sigC)
                nc.vector.tensor_add(
                    out=acc[:p, j : j + 1], in0=sigB[:p, :], in1=sigA[:p, :]
                )

            nc.sync.dma_start(out=out_d.ap(), in_=acc[:, :])

    nc.compile()
    return nc


def run(inputs, targets, groups, trace=False):
    """Returns (loss, exec_time_ns or None)."""
    from concourse import bass_utils

    B = inputs.shape[0]
    assert inputs.shape[1] == C and B % N_CORES == 0
    rows = B // N_CORES

    key = (rows, tuple(int(v) for v in groups))
    if key not in _prog_cache:
        _prog_cache[key] = build_program(rows, key[1])
    nc = _prog_cache[key]

    x = np.ascontiguousarray(inputs, dtype=np.float32)
    t = np.ascontiguousarray(targets, dtype=np.float32)
    in_maps = [
        {
            "x": x[c * rows : (c + 1) * rows],
            "t": t[c * rows : (c + 1) * rows],
        }
        for c in range(N_CORES)
    ]
    res = bass_utils.run_bass_kernel_spmd(
        nc, in_maps, core_ids=list(range(N_CORES)), trace=trace
    )
    total = sum(float(r["out"].astype(np.float64).sum()) for r in res.results)
    return np.float32(total / (B * C)), res.exec_time_ns


def kernel(inputs, targets, groups):
    return run(inputs, targets, groups)[0]
